# revision 80
# baseline (speedup 1.0000x reference)
"""GCN layer kernel for TRN2, data-parallel over batch across 8 NeuronCores.

Per core (one batch b) the GCN branch runs in fp8-e4m3 (it contributes
~0.1% of the output magnitude; the bf16 residual + fp32-stats LN path
dominates accuracy):

  load:   one SP DMA queue: validrep, adjT fp8 as 8 resident j-pair
          tiles [128, 2, 2048] (DoubleRow layout), small params, x bf16
          pairs, W fp8.  Total HBM traffic ~10.3 MiB vs 32 MiB for the
          f32 two-pass version.
  deg:    replicated-valid stationary -> fp8 DoubleRow matvecs give deg
          rows broadcast to all partitions as adj pairs land; a 16x
          transpose column path yields dis/u/c1/c2 [128,16] columns.
  x side: z = u*x (fp8 pairs, DVE/Act), xc2 = c2*x (bf16, Pool).
  B:      mm[i,d] = sum_j adjT[j,i] z[j,d], one fp8 DoubleRow accum per
          i-block (output in ROW layout so c1 is a per-partition
          scalar); epilogue is a single DVE STT:
          agg_row = mm*c1 + xc2.  PE-transpose + Act copy pack agg into
          agg_big [d, m, i] fp8 for phase C.
  C:      out2 = aggT.T @ W.T (fp8 DoubleRow, d-block-pair tiles);
          relu/scale on Act, residual + moment accumulation via
          STT/Square-accum, packed [128,4] LN stats on DVE, t1 split
          Pool/DVE, stores on the idle SP queue.  C groups are emitted
          2 B-blocks behind to keep epilogues ahead in engine queues.

Act tables (Sqrt/Square/Relu) are preloaded during the DMA prefix.
"""
import os
import numpy as np
import ml_dtypes

import concourse.bacc as bacc
import concourse.tile as tile
import concourse.mybir as mybir
from concourse.bass_utils import run_bass_kernel_spmd

B, L, D = 8, 2048, 512
JBN = L // 128      # 16 row blocks
JPN = JBN // 2      # 8 j-pairs (DoubleRow)
NCH = L // 512      # 4 i-chunks of 512
DBN = D // 128      # 4 d-blocks
LN_EPS = 1e-5
DSCALE = float(D) ** -0.5
F32 = mybir.dt.float32
F32R = mybir.dt.float32r
BF16 = mybir.dt.bfloat16
FP8 = mybir.dt.float8e4
MUL = mybir.AluOpType.mult
ADD = mybir.AluOpType.add
SUB = mybir.AluOpType.subtract
DR = mybir.MatmulPerfMode.DoubleRow
NPF8 = ml_dtypes.float8_e4m3

LAST_RESULT = None  # BassKernelResults of the most recent run (for profiling)


def _build_program(ln_identity=False, bias_zero=False):
    nc = bacc.Bacc("TRN2", target_bir_lowering=False, debug=False)
    d = {}
    def di(name, shape, dt):
        d[name] = nc.dram_tensor(name, shape, dt, kind="ExternalInput").ap()
    di("adj8", [JPN, 128, 2, L], FP8)      # j-pair DoubleRow layout
    di("x_in", [128, JPN, 2, D], BF16)     # same pairing for x
    di("validrep8", [128, JPN, 2, 128], FP8)  # valid replicated along M
    di("validc_f", [128, JBN], F32)
    di("ewc", [128, 1], F32)
    di("wt8", [2, 128, 2, D], FP8)         # W.T as d-block-pair tiles
    di("b_row_r", [1, D], F32R)
    di("ones_row", [1, 128], F32R)
    di("lnw_row", [1, D], F32)
    di("lnb_row", [1, D], F32)
    di("ident", [128, 128], F32)
    di("identh", [128, 128], BF16)
    out_d = nc.dram_tensor("out_t", [L, D], F32, kind="ExternalOutput").ap()

    with tile.TileContext(nc) as tc:
        with tc.tile_pool(name="pAdj", bufs=JPN) as pAdj, \
             tc.tile_pool(name="pX", bufs=JPN) as pX, \
             tc.tile_pool(name="pZ", bufs=JPN) as pZ, \
             tc.tile_pool(name="pXT", bufs=JBN) as pXT, \
             tc.tile_pool(name="pAgg", bufs=1) as pAgg, \
             tc.tile_pool(name="pW", bufs=2) as pW, \
             tc.tile_pool(name="pCol", bufs=24) as pCol, \
             tc.tile_pool(name="pSmall", bufs=1) as pSmall:

            # persistent arrays
            adj_t = [pAdj.tile([128, 2, L], FP8, tag="adj", name=f"adj{q}")
                     for q in range(JPN)]
            xbig = pX.tile([128, JPN, 2, D], BF16, tag="x", name="xbig",
                           bufs=1)
            x_t = [xbig[:, q, :, :] for q in range(JPN)]
            z_t = [pZ.tile([128, 2, D], FP8, tag="z", name=f"z{q}")
                   for q in range(JPN)]
            xc2_t = [pXT.tile([128, D], BF16, tag="xc2", name=f"xc2r{jb}")
                     for jb in range(JBN)]
            # aggT as one tile [d_part, m, i]; C lhsT slices d-block pairs
            agg_big = pAgg.tile([128, DBN, L], FP8, tag="agg", name="aggbig",
                                bufs=1)
            wt_t = [pW.tile([128, 2, D], FP8, tag="wt", name=f"wt{k2}")
                    for k2 in range(2)]
            eps_t = pSmall.tile([128, 1], F32, tag="eps")
            nc.vector.memset(eps_t[:], LN_EPS)
            # preload act tables (Sqrt/Square/Relu) during the idle prefix
            warm_t = pSmall.tile([128, 1], F32, tag="warm")
            nc.scalar.sqrt(warm_t[:], eps_t[:])
            nc.scalar.activation(warm_t[:], eps_t[:],
                                 mybir.ActivationFunctionType.Square)
            nc.scalar.activation(warm_t[:], eps_t[:],
                                 mybir.ActivationFunctionType.Relu)
            ones_t = pSmall.tile([1, 128], F32R, tag="ones")
            browr_t = pSmall.tile([1, D], F32R, tag="browr")
            stat_b = {}

            # ---- input streaming, all on the SP (sync) queue in order ----
            # tiny params first, then adj pairs (deg as they land), then x
            vrep_t = pSmall.tile([128, JPN, 2, 128], FP8, tag="vr",
                                 name="vrep")
            nc.sync.dma_start(vrep_t[:], d["validrep8"][:])
            for q in range(JPN):
                nc.sync.dma_start(adj_t[q][:], d["adj8"][q, :, :, :])
            validf_t = pSmall.tile([128, JBN], F32, tag="vf")
            nc.sync.dma_start(validf_t[:], d["validc_f"][:])
            ewc_t = pSmall.tile([128, 1], F32, tag="ew")
            nc.sync.dma_start(ewc_t[:], d["ewc"][:])
            ident_t = pSmall.tile([128, 128], F32, tag="ident")
            nc.sync.dma_start(ident_t[:], d["ident"][:])
            for q in range(JPN):
                nc.sync.dma_start(xbig[:, q:q + 1, :, :],
                                  d["x_in"][:, q:q + 1, :, :])
            identh_t = pSmall.tile([128, 128], BF16, tag="identh")
            nc.sync.dma_start(identh_t[:], d["identh"][:])
            for k2 in range(2):
                nc.sync.dma_start(wt_t[k2][:], d["wt8"][k2, :, :, :])
            nc.sync.dma_start(ones_t[:], d["ones_row"][:])
            nc.sync.dma_start(browr_t[:], d["b_row_r"][:])
            rows = {}
            for nm in ("lnw_row", "lnb_row"):
                r = pSmall.tile([1, D], F32, tag=nm, name=nm + "_t")
                nc.sync.dma_start(r[:], d[nm][:])
                rows[nm] = r



            if not ln_identity:
                for nm in ("lnw_row", "lnb_row"):
                    t = pSmall.tile([128, D], F32, tag=nm + "b", name=nm + "_b")
                    nc.gpsimd.partition_broadcast(t[:], rows[nm][:])
                    stat_b[nm] = t

            with tc.tile_pool(name="psA", bufs=1, space="PSUM") as psA, \
                 tc.tile_pool(name="psB", bufs=5, space="PSUM") as psB, \
                 tc.tile_pool(name="psC", bufs=2, space="PSUM") as psC, \
                 tc.tile_pool(name="pScr", bufs=8) as pScr, \
                 tc.tile_pool(name="pOut", bufs=8) as pOut:

                # ---- deg matvecs: replicated-valid stationary gives deg
                # rows broadcast to all 128 partitions (no later broadcast)
                dps = [psB.tile([128, 512], F32, tag="mm", name=f"dps{c}")
                       for c in range(NCH)]
                for q in range(JPN):
                    for c in range(NCH):
                        nc.tensor.matmul(
                            dps[c][:],
                            vrep_t[:, q, :, :],
                            adj_t[q][:, :, c * 512:(c + 1) * 512],
                            start=(q == 0), stop=(q == JPN - 1),
                            perf_mode=DR)

                # stage deg rows to SBUF for the column-path transposes
                tcs = []
                for c in range(NCH):
                    t_c = pScr.tile([1, 512], F32, tag="crow",
                                    name=f"tc{c}", bufs=4)
                    if c % 2 == 0:
                        nc.vector.tensor_copy(t_c[:], dps[c][0:1, :])
                    else:
                        nc.scalar.copy(t_c[:], dps[c][0:1, :])
                    tcs.append(t_c)

                # column path: transpose raw deg rows -> [128, JBN]
                rc_ps = psA.tile([128, JBN], F32, tag="deg", name="rc_ps")
                for v in range(JBN):
                    c, w = v // 4, v % 4
                    nc.tensor.transpose(
                        rc_ps[:, v:v + 1],
                        tcs[c][0:1, w * 128:(w + 1) * 128],
                        ident_t[0:1, 0:1])
                mdeg_col = pCol.tile([128, JBN], F32, tag="degc", bufs=1)
                nc.vector.scalar_tensor_tensor(
                    mdeg_col[:], rc_ps[:], 1.0, validf_t[:], MUL, MUL)
                std_col = pCol.tile([128, JBN], F32, tag="stdc", bufs=1)
                nc.scalar.activation(
                    std_col[:], mdeg_col[:],
                    mybir.ActivationFunctionType.Sqrt, bias=1.0)
                dis_col = pCol.tile([128, JBN], F32, tag="disc", bufs=1)
                nc.vector.reciprocal(dis_col[:], std_col[:])
                # u = dis * valid; c1 = ew*u; c2 = ew*dis^2
                u_col = pCol.tile([128, JBN], F32, tag="uc", bufs=1)
                nc.vector.tensor_mul(u_col[:], dis_col[:], validf_t[:])
                c1_col = pCol.tile([128, JBN], F32, tag="c1c", bufs=1)
                nc.vector.tensor_scalar_mul(c1_col[:], u_col[:], ewc_t[:])
                c2_col = pCol.tile([128, JBN], F32, tag="c2c", bufs=1)
                nc.vector.scalar_tensor_tensor(
                    c2_col[:], dis_col[:], ewc_t[:], dis_col[:], MUL, MUL)

                # ---- per x pair: z (fp8) + xc2 row (bf16) ----
                for q in range(JPN):
                    for s in range(2):
                        jb = 2 * q + s
                        if s == 0:
                            nc.vector.tensor_scalar_mul(
                                z_t[q][:, s, :], x_t[q][:, s, :],
                                u_col[:, jb:jb + 1])
                        else:
                            nc.scalar.mul(z_t[q][:, s, :], x_t[q][:, s, :],
                                          u_col[:, jb:jb + 1])
                        nc.gpsimd.tensor_scalar_mul(
                            xc2_t[jb][:], x_t[q][:, s, :],
                            c2_col[:, jb:jb + 1])

                # ---- B per i-block: mm[i, d] = sum_j adjT[j,i] z[j,d],
                # epilogue STT folds c1 (partition scalar) and the x self
                # loop, then PE transposes into agg_big [d, m, i]
                for lb in range(JBN):
                    mmB = psB.tile([128, 512], F32, tag="mm",
                                   name=f"mmB{lb}")
                    for q in range(JPN):
                        nc.tensor.matmul(
                            mmB[:],
                            adj_t[q][:, :, lb * 128:(lb + 1) * 128],
                            z_t[q][:, :, :],
                            start=(q == 0), stop=(q == JPN - 1),
                            perf_mode=DR)
                    aggr = pScr.tile([128, D], BF16, tag="aggr", bufs=4,
                                     name=f"aggr{lb}")
                    nc.vector.scalar_tensor_tensor(
                        aggr[:], mmB[:], c1_col[:, lb:lb + 1],
                        xc2_t[lb][:], MUL, ADD)
                    pt = psA.tile([128, DBN, 128], BF16, tag="deg",
                                  name=f"ptb{lb}")
                    for m in range(DBN):
                        nc.tensor.transpose(
                            pt[:, m, :], aggr[:, m * 128:(m + 1) * 128],
                            identh_t[:])
                    nc.scalar.copy(
                        agg_big[:, :, lb * 128:(lb + 1) * 128], pt[:])

                    # C groups staggered 2 blocks behind B to keep the
                    # next wave's epilogues ahead in the engine queues
                    if lb not in (5, 9, 13, 15):
                        continue
                    groups = [(lb - 5) // 4] if lb != 15 else [3]
                    for p in groups:
                        emit_c_group(p)

                def _unused():
                    p = 0
                    lbs = list(range(4 * p, 4 * (p + 1)))
                    ps2d, rd, hhd = {}, {}, {}
                    sums4 = pCol.tile([128, 4], F32, tag="lncol",
                                      name=f"su4_{p}")
                    m2s4 = pCol.tile([128, 4], F32, tag="lncol",
                                     name=f"m2s4_{p}")
                    for j, lb in enumerate(lbs):
                        off = lb * 128
                        ps2 = psC.tile([128, D], F32, tag="mmc",
                                       name=f"mm2_{lb}")
                        for k2 in range(2):
                            nc.tensor.matmul(
                                ps2[:],
                                agg_big[:, 2 * k2:2 * k2 + 2,
                                        off:off + 128],
                                wt_t[k2][:],
                                start=(k2 == 0),
                                stop=(bias_zero and k2 == 1),
                                perf_mode=DR)
                        if not bias_zero:
                            nc.tensor.matmul(ps2[:], ones_t[:], browr_t[:],
                                             start=False, stop=True)
                        ps2d[lb] = ps2
                    for j, lb in enumerate(lbs):
                        r = pScr.tile([128, D], BF16, tag="relu", bufs=6,
                                      name=f"r{lb}")
                        nc.scalar.activation(
                            r[:], ps2d[lb][:],
                            mybir.ActivationFunctionType.Relu,
                            scale=DSCALE)
                        rd[lb] = r
                    for j, lb in enumerate(lbs):
                        hh = pScr.tile([128, D], BF16, tag="hh", bufs=10,
                                       name=f"hh{lb}")
                        xblk = x_t[lb // 2][:, lb % 2, :]
                        nc.vector.scalar_tensor_tensor(
                            hh[:], rd[lb][:], 1.0, xblk, MUL, ADD,
                            accum_out=sums4[:, j:j + 1])
                        hhd[lb] = hh
                    for j, lb in enumerate(lbs):
                        sq = pScr.tile([128, D], BF16, tag="sq", bufs=4,
                                       name=f"sq{lb}")
                        if lb % 2 == 0:
                            nc.scalar.activation(
                                sq[:], hhd[lb][:],
                                mybir.ActivationFunctionType.Square,
                                accum_out=m2s4[:, j:j + 1])
                        else:
                            nc.vector.scalar_tensor_tensor(
                                sq[:], hhd[lb][:], 1.0, hhd[lb][:],
                                MUL, MUL, accum_out=m2s4[:, j:j + 1])
                    # packed LN stats for the 4 blocks (DVE-local chain)
                    mu4 = pCol.tile([128, 4], F32, tag="lncol",
                                    name=f"mu4_{p}")
                    nc.vector.tensor_scalar_mul(mu4[:], sums4[:], 1.0 / D)
                    m2n4 = pCol.tile([128, 4], F32, tag="lncol",
                                     name=f"m2n4_{p}")
                    nc.vector.tensor_scalar_mul(m2n4[:], m2s4[:], 1.0 / D)
                    sqmu4 = pCol.tile([128, 4], F32, tag="lncol",
                                      name=f"sqmu4_{p}")
                    nc.vector.tensor_mul(sqmu4[:], mu4[:], mu4[:])
                    negv4 = pCol.tile([128, 4], F32, tag="lncol",
                                      name=f"negv4_{p}")
                    nc.vector.tensor_sub(negv4[:], sqmu4[:], m2n4[:])
                    stdt4 = pCol.tile([128, 4], F32, tag="lncol",
                                      name=f"stdt4_{p}")
                    nc.scalar.activation(
                        stdt4[:], negv4[:],
                        mybir.ActivationFunctionType.Sqrt,
                        scale=-1.0, bias=eps_t[:])
                    rstd4 = pCol.tile([128, 4], F32, tag="lncol",
                                      name=f"rstd4_{p}")
                    nc.vector.reciprocal(rstd4[:], stdt4[:])
                    for j, lb in enumerate(lbs):
                        eng1 = nc.gpsimd if lb % 2 == 0 else nc.vector
                        t1 = pOut.tile([128, D], F32, tag="o",
                                       name=f"t1{lb}")
                        eng1.tensor_scalar(t1[:], hhd[lb][:],
                                           mu4[:, j:j + 1],
                                           rstd4[:, j:j + 1], SUB, MUL)
                        if ln_identity:
                            nc.sync.dma_start(
                                out_d[lb * 128:(lb + 1) * 128, :], t1[:])
                        else:
                            tt = pScr.tile([128, D], F32, tag="scr",
                                           name=f"tt{lb}")
                            teng = nc.vector if lb % 2 == 0 else nc.gpsimd
                            teng.tensor_mul(tt[:], t1[:],
                                            stat_b["lnw_row"][:])
                            o_sb = pOut.tile([128, D], F32, tag="o",
                                             name=f"o{lb}")
                            nc.gpsimd.tensor_add(o_sb[:], tt[:],
                                                 stat_b["lnb_row"][:])
                            nc.sync.dma_start(
                                out_d[lb * 128:(lb + 1) * 128, :],
                                o_sb[:])

    nc.compile()
    return nc


_NC_CACHE = {}


def _get_nc(ln_identity=False, bias_zero=False):
    key = (ln_identity, bias_zero)
    if key not in _NC_CACHE:
        _NC_CACHE[key] = _build_program(*key)
    return _NC_CACHE[key]


def kernel(x, adj, pad_mask, W, b, ln_w, ln_b, edge_weight):
    global LAST_RESULT
    x = np.asarray(x, dtype=np.float32)
    adj = np.asarray(adj, dtype=np.float32)
    pad_mask = np.asarray(pad_mask)
    W = np.asarray(W, dtype=np.float32)
    b = np.asarray(b, dtype=np.float32)
    ln_w = np.asarray(ln_w, dtype=np.float32)
    ln_b = np.asarray(ln_b, dtype=np.float32)
    ew = float(np.asarray(edge_weight).reshape(-1)[0])

    ln_identity = bool(np.all(ln_w == 1.0) and np.all(ln_b == 0.0))
    bias_zero = bool(np.all(b == 0.0))
    nc = _get_nc(ln_identity, bias_zero)

    # W.T in d-block-pair DoubleRow layout [2, 128, 2, D]
    wt8 = np.ascontiguousarray(W.T).astype(NPF8)          # [D, D] = [d, o]
    wt8 = np.ascontiguousarray(
        wt8.reshape(2, 2, 128, D).transpose(0, 2, 1, 3))  # [k2, p, s, o]
    ewc = np.full((128, 1), ew, dtype=np.float32)
    ident = np.eye(128, dtype=np.float32)
    b_row_r = b.reshape(1, D).copy()
    ones_row = np.ones((1, 128), dtype=np.float32)
    lnw_row = np.ascontiguousarray(ln_w.reshape(1, D))
    lnb_row = np.ascontiguousarray(ln_b.reshape(1, D))

    in_maps = []
    for c in range(B):
        adjT8 = np.ascontiguousarray(adj[c].T).astype(NPF8)   # [j, i]
        adj8 = np.ascontiguousarray(
            adjT8.reshape(JPN, 2, 128, L).transpose(0, 2, 1, 3))
        x8 = np.ascontiguousarray(
            x[c].reshape(JPN, 2, 128, D).transpose(2, 0, 1, 3)).astype(
                ml_dtypes.bfloat16)
        valid = (~pad_mask[c]).astype(np.float32)
        validc = np.ascontiguousarray(valid.reshape(JBN, 128).T)
        # [128, JPN, 2, 128]: valid[(2q+s)*128+p] replicated along last axis
        vrep = np.broadcast_to(
            valid.reshape(JPN, 2, 128).transpose(2, 0, 1)[:, :, :, None],
            (128, JPN, 2, 128))
        in_maps.append({
            "adj8": adj8,
            "x_in": x8,
            "validrep8": np.ascontiguousarray(vrep).astype(NPF8),
            "validc_f": validc,
            "ewc": ewc,
            "wt8": wt8,
            "b_row_r": b_row_r,
            "ones_row": ones_row,
            "lnw_row": lnw_row,
            "lnb_row": lnb_row,
            "ident": ident,
            "identh": ident.astype(ml_dtypes.bfloat16),
        })

    trace = os.environ.get("KERNEL_TRACE", "0") == "1"
    res = run_bass_kernel_spmd(nc, in_maps, core_ids=list(range(B)), trace=trace)
    LAST_RESULT = res
    out = np.stack([res.results[c]["out_t"] for c in range(B)], axis=0)
    return out


# revision 88
# speedup vs baseline: 1.0024x; 1.0024x over previous
"""GCN layer kernel for TRN2, data-parallel over batch across 8 NeuronCores.

Per core (one batch b) the GCN branch runs in fp8-e4m3 (it contributes
~0.1% of the output magnitude; the bf16 residual + fp32-stats LN path
dominates accuracy):

  load:   one SP DMA queue: validrep, adjT fp8 as 8 resident j-pair
          tiles [128, 2, 2048] (DoubleRow layout), small params, x bf16
          pairs, W fp8.  Total HBM traffic ~10.3 MiB vs 32 MiB for the
          f32 two-pass version.
  deg:    replicated-valid stationary -> fp8 DoubleRow matvecs give deg
          rows broadcast to all partitions as adj pairs land; a 16x
          transpose column path yields dis/u/c1/c2 [128,16] columns.
  x side: z = u*x (fp8 pairs, DVE/Act), xc2 = c2*x (bf16, Pool).
  B:      mm[i,d] = sum_j adjT[j,i] z[j,d], one fp8 DoubleRow accum per
          i-block (output in ROW layout so c1 is a per-partition
          scalar); epilogue is a single DVE STT:
          agg_row = mm*c1 + xc2.  PE-transpose + Act copy pack agg into
          agg_big [d, m, i] fp8 for phase C.
  C:      out2 = aggT.T @ W.T (fp8 DoubleRow, d-block-pair tiles);
          relu/scale on Act, residual + moment accumulation via
          STT/Square-accum, packed [128,4] LN stats on DVE, t1 split
          Pool/DVE, stores on the idle SP queue.  C groups are emitted
          2 B-blocks behind to keep epilogues ahead in engine queues.

Act tables (Sqrt/Square/Relu) are preloaded during the DMA prefix.
"""
import os
import numpy as np
import ml_dtypes

import concourse.bacc as bacc
import concourse.tile as tile
import concourse.mybir as mybir
from concourse.bass_utils import run_bass_kernel_spmd

B, L, D = 8, 2048, 512
JBN = L // 128      # 16 row blocks
JPN = JBN // 2      # 8 j-pairs (DoubleRow)
NCH = L // 512      # 4 i-chunks of 512
DBN = D // 128      # 4 d-blocks
LN_EPS = 1e-5
DSCALE = float(D) ** -0.5
F32 = mybir.dt.float32
F32R = mybir.dt.float32r
BF16 = mybir.dt.bfloat16
FP8 = mybir.dt.float8e4
MUL = mybir.AluOpType.mult
ADD = mybir.AluOpType.add
SUB = mybir.AluOpType.subtract
DR = mybir.MatmulPerfMode.DoubleRow
NPF8 = ml_dtypes.float8_e4m3

LAST_RESULT = None  # BassKernelResults of the most recent run (for profiling)


def _build_program(ln_identity=False, bias_zero=False):
    nc = bacc.Bacc("TRN2", target_bir_lowering=False, debug=False)
    d = {}
    def di(name, shape, dt):
        d[name] = nc.dram_tensor(name, shape, dt, kind="ExternalInput").ap()
    di("adj8", [JPN, 128, 2, L], FP8)      # j-pair DoubleRow layout
    di("x_in", [128, JPN, 2, D], BF16)     # same pairing for x
    di("validrep8", [128, JPN, 2, 128], FP8)  # valid replicated along M
    di("validc_f", [128, JBN], F32)
    di("ewc", [128, 1], F32)
    di("wt8", [2, 128, 2, D], FP8)         # W.T as d-block-pair tiles
    di("b_row_r", [1, D], F32R)
    di("ones_row", [1, 128], F32R)
    di("lnw_row", [1, D], F32)
    di("lnb_row", [1, D], F32)
    di("ident", [128, 128], F32)
    di("identh", [128, 128], BF16)
    out_d = nc.dram_tensor("out_t", [L, D], F32, kind="ExternalOutput").ap()

    with tile.TileContext(nc) as tc:
        with tc.tile_pool(name="pAdj", bufs=JPN) as pAdj, \
             tc.tile_pool(name="pX", bufs=JPN) as pX, \
             tc.tile_pool(name="pZ", bufs=JPN) as pZ, \
             tc.tile_pool(name="pXT", bufs=JBN) as pXT, \
             tc.tile_pool(name="pAgg", bufs=1) as pAgg, \
             tc.tile_pool(name="pW", bufs=2) as pW, \
             tc.tile_pool(name="pCol", bufs=24) as pCol, \
             tc.tile_pool(name="pSmall", bufs=1) as pSmall:

            # persistent arrays
            adj_t = [pAdj.tile([128, 2, L], FP8, tag="adj", name=f"adj{q}")
                     for q in range(JPN)]
            xbig = pX.tile([128, JPN, 2, D], BF16, tag="x", name="xbig",
                           bufs=1)
            x_t = [xbig[:, q, :, :] for q in range(JPN)]
            z_t = [pZ.tile([128, 2, D], FP8, tag="z", name=f"z{q}")
                   for q in range(JPN)]
            xc2_t = [pXT.tile([128, D], BF16, tag="xc2", name=f"xc2r{jb}")
                     for jb in range(JBN)]
            # aggT as one tile [d_part, m, i]; C lhsT slices d-block pairs
            agg_big = pAgg.tile([128, DBN, L], FP8, tag="agg", name="aggbig",
                                bufs=1)
            wt_t = [pW.tile([128, 2, D], FP8, tag="wt", name=f"wt{k2}")
                    for k2 in range(2)]
            eps_t = pSmall.tile([128, 1], F32, tag="eps")
            nc.vector.memset(eps_t[:], LN_EPS * D)
            # preload act tables (Sqrt/Square/Relu) during the idle prefix
            warm_t = pSmall.tile([128, 1], F32, tag="warm")
            nc.scalar.sqrt(warm_t[:], eps_t[:])
            nc.scalar.activation(warm_t[:], eps_t[:],
                                 mybir.ActivationFunctionType.Square)
            nc.scalar.activation(warm_t[:], eps_t[:],
                                 mybir.ActivationFunctionType.Relu)
            ones_t = pSmall.tile([1, 128], F32R, tag="ones")
            browr_t = pSmall.tile([1, D], F32R, tag="browr")
            stat_b = {}

            # ---- input streaming, all on the SP (sync) queue in order ----
            # tiny params first, then adj pairs (deg as they land), then x
            vrep_t = pSmall.tile([128, JPN, 2, 128], FP8, tag="vr",
                                 name="vrep")
            nc.sync.dma_start(vrep_t[:], d["validrep8"][:])
            for q in range(JPN):
                nc.sync.dma_start(adj_t[q][:], d["adj8"][q, :, :, :])
            validf_t = pSmall.tile([128, JBN], F32, tag="vf")
            nc.sync.dma_start(validf_t[:], d["validc_f"][:])
            ewc_t = pSmall.tile([128, 1], F32, tag="ew")
            nc.sync.dma_start(ewc_t[:], d["ewc"][:])
            ident_t = pSmall.tile([128, 128], F32, tag="ident")
            nc.sync.dma_start(ident_t[:], d["ident"][:])
            for q in range(JPN):
                nc.sync.dma_start(xbig[:, q:q + 1, :, :],
                                  d["x_in"][:, q:q + 1, :, :])
            identh_t = pSmall.tile([128, 128], BF16, tag="identh")
            nc.sync.dma_start(identh_t[:], d["identh"][:])
            for k2 in range(2):
                nc.sync.dma_start(wt_t[k2][:], d["wt8"][k2, :, :, :])
            nc.sync.dma_start(ones_t[:], d["ones_row"][:])
            nc.sync.dma_start(browr_t[:], d["b_row_r"][:])
            rows = {}
            for nm in ("lnw_row", "lnb_row"):
                r = pSmall.tile([1, D], F32, tag=nm, name=nm + "_t")
                nc.sync.dma_start(r[:], d[nm][:])
                rows[nm] = r



            if not ln_identity:
                for nm in ("lnw_row", "lnb_row"):
                    t = pSmall.tile([128, D], F32, tag=nm + "b", name=nm + "_b")
                    nc.gpsimd.partition_broadcast(t[:], rows[nm][:])
                    stat_b[nm] = t

            with tc.tile_pool(name="psA", bufs=1, space="PSUM") as psA, \
                 tc.tile_pool(name="psB", bufs=5, space="PSUM") as psB, \
                 tc.tile_pool(name="psC", bufs=2, space="PSUM") as psC, \
                 tc.tile_pool(name="pScr", bufs=8) as pScr, \
                 tc.tile_pool(name="pOut", bufs=8) as pOut:

                # ---- deg matvecs: replicated-valid stationary gives deg
                # rows broadcast to all 128 partitions (no later broadcast)
                dps = [psB.tile([128, 512], F32, tag="mm", name=f"dps{c}")
                       for c in range(NCH)]
                for q in range(JPN):
                    for c in range(NCH):
                        nc.tensor.matmul(
                            dps[c][:],
                            vrep_t[:, q, :, :],
                            adj_t[q][:, :, c * 512:(c + 1) * 512],
                            start=(q == 0), stop=(q == JPN - 1),
                            perf_mode=DR)

                # stage deg rows to SBUF for the column-path transposes
                tcs = []
                for c in range(NCH):
                    t_c = pScr.tile([1, 512], F32, tag="crow",
                                    name=f"tc{c}", bufs=4)
                    if c % 2 == 0:
                        nc.vector.tensor_copy(t_c[:], dps[c][0:1, :])
                    else:
                        nc.scalar.copy(t_c[:], dps[c][0:1, :])
                    tcs.append(t_c)

                # column path: transpose raw deg rows -> [128, JBN]
                rc_ps = psA.tile([128, JBN], F32, tag="deg", name="rc_ps")
                for v in range(JBN):
                    c, w = v // 4, v % 4
                    nc.tensor.transpose(
                        rc_ps[:, v:v + 1],
                        tcs[c][0:1, w * 128:(w + 1) * 128],
                        ident_t[0:1, 0:1])
                mdeg_col = pCol.tile([128, JBN], F32, tag="degc", bufs=1)
                nc.vector.scalar_tensor_tensor(
                    mdeg_col[:], rc_ps[:], 1.0, validf_t[:], MUL, MUL)
                std_col = pCol.tile([128, JBN], F32, tag="stdc", bufs=1)
                nc.scalar.activation(
                    std_col[:], mdeg_col[:],
                    mybir.ActivationFunctionType.Sqrt, bias=1.0)
                dis_col = pCol.tile([128, JBN], F32, tag="disc", bufs=1)
                nc.vector.reciprocal(dis_col[:], std_col[:])
                # u = dis * valid; c1 = ew*u; c2 = ew*dis^2
                u_col = pCol.tile([128, JBN], F32, tag="uc", bufs=1)
                nc.vector.tensor_mul(u_col[:], dis_col[:], validf_t[:])
                c1_col = pCol.tile([128, JBN], F32, tag="c1c", bufs=1)
                nc.vector.tensor_scalar_mul(c1_col[:], u_col[:], ewc_t[:])
                c2_col = pCol.tile([128, JBN], F32, tag="c2c", bufs=1)
                nc.vector.scalar_tensor_tensor(
                    c2_col[:], dis_col[:], ewc_t[:], dis_col[:], MUL, MUL)

                # ---- per x pair: z (fp8) + xc2 row (bf16) ----
                for q in range(JPN):
                    for s in range(2):
                        jb = 2 * q + s
                        if s == 0:
                            nc.vector.tensor_scalar_mul(
                                z_t[q][:, s, :], x_t[q][:, s, :],
                                u_col[:, jb:jb + 1])
                        else:
                            nc.scalar.mul(z_t[q][:, s, :], x_t[q][:, s, :],
                                          u_col[:, jb:jb + 1])
                        nc.gpsimd.tensor_scalar_mul(
                            xc2_t[jb][:], x_t[q][:, s, :],
                            c2_col[:, jb:jb + 1])

                # ---- B per i-block: mm[i, d] = sum_j adjT[j,i] z[j,d],
                # epilogue STT folds c1 (partition scalar) and the x self
                # loop, then PE transposes into agg_big [d, m, i]
                for lb in range(JBN):
                    mmB = psB.tile([128, 512], F32, tag="mm",
                                   name=f"mmB{lb}")
                    for q in range(JPN):
                        nc.tensor.matmul(
                            mmB[:],
                            adj_t[q][:, :, lb * 128:(lb + 1) * 128],
                            z_t[q][:, :, :],
                            start=(q == 0), stop=(q == JPN - 1),
                            perf_mode=DR)
                    aggr = pScr.tile([128, D], BF16, tag="aggr", bufs=4,
                                     name=f"aggr{lb}")
                    if lb >= 14:
                        s1 = pScr.tile([128, D], BF16, tag="s1", bufs=2,
                                       name=f"s1_{lb}")
                        nc.scalar.mul(s1[:], mmB[:], c1_col[:, lb:lb + 1])
                        nc.gpsimd.tensor_add(aggr[:], s1[:], xc2_t[lb][:])
                    else:
                        nc.vector.scalar_tensor_tensor(
                            aggr[:], mmB[:], c1_col[:, lb:lb + 1],
                            xc2_t[lb][:], MUL, ADD)
                    pt = psA.tile([128, DBN, 128], BF16, tag="deg",
                                  name=f"ptb{lb}")
                    for m in range(DBN):
                        nc.tensor.transpose(
                            pt[:, m, :], aggr[:, m * 128:(m + 1) * 128],
                            identh_t[:])
                    nc.scalar.copy(
                        agg_big[:, :, lb * 128:(lb + 1) * 128], pt[:])

                    # C groups staggered 2 blocks behind B to keep the
                    # next wave's epilogues ahead in the engine queues
                    if lb not in (5, 9, 13, 15):
                        continue
                    groups = [(lb - 5) // 4] if lb != 15 else [3]
                    for p in groups:
                        emit_c_group(p)

                def _unused():
                    p = 0
                    lbs = list(range(4 * p, 4 * (p + 1)))
                    ps2d, rd, hhd = {}, {}, {}
                    sums4 = pCol.tile([128, 4], F32, tag="lncol",
                                      name=f"su4_{p}")
                    m2s4 = pCol.tile([128, 4], F32, tag="lncol",
                                     name=f"m2s4_{p}")
                    for j, lb in enumerate(lbs):
                        off = lb * 128
                        ps2 = psC.tile([128, D], F32, tag="mmc",
                                       name=f"mm2_{lb}")
                        for k2 in range(2):
                            nc.tensor.matmul(
                                ps2[:],
                                agg_big[:, 2 * k2:2 * k2 + 2,
                                        off:off + 128],
                                wt_t[k2][:],
                                start=(k2 == 0),
                                stop=(bias_zero and k2 == 1),
                                perf_mode=DR)
                        if not bias_zero:
                            nc.tensor.matmul(ps2[:], ones_t[:], browr_t[:],
                                             start=False, stop=True)
                        ps2d[lb] = ps2
                    # x is host-scaled by sqrt(D) and ewc by D^-0.5, so
                    # LN(hh') == LN(h): relu+scale+residual fuse into one STT
                    for j, lb in enumerate(lbs):
                        hh = pScr.tile([128, D], BF16, tag="hh", bufs=10,
                                       name=f"hh{lb}")
                        xblk = x_t[lb // 2][:, lb % 2, :]
                        nc.vector.scalar_tensor_tensor(
                            hh[:], ps2d[lb][:], 0.0, xblk,
                            mybir.AluOpType.max, ADD,
                            accum_out=sums4[:, j:j + 1])
                        hhd[lb] = hh
                    for j, lb in enumerate(lbs):
                        sq = pScr.tile([128, D], BF16, tag="sq", bufs=4,
                                       name=f"sq{lb}")
                        nc.scalar.activation(
                            sq[:], hhd[lb][:],
                            mybir.ActivationFunctionType.Square,
                            accum_out=m2s4[:, j:j + 1])
                    # packed LN stats for the 4 blocks (DVE-local chain)
                    mu4 = pCol.tile([128, 4], F32, tag="lncol",
                                    name=f"mu4_{p}")
                    nc.vector.tensor_scalar_mul(mu4[:], sums4[:], 1.0 / D)
                    m2n4 = pCol.tile([128, 4], F32, tag="lncol",
                                     name=f"m2n4_{p}")
                    nc.vector.tensor_scalar_mul(m2n4[:], m2s4[:], 1.0 / D)
                    sqmu4 = pCol.tile([128, 4], F32, tag="lncol",
                                      name=f"sqmu4_{p}")
                    nc.vector.tensor_mul(sqmu4[:], mu4[:], mu4[:])
                    negv4 = pCol.tile([128, 4], F32, tag="lncol",
                                      name=f"negv4_{p}")
                    nc.vector.tensor_sub(negv4[:], sqmu4[:], m2n4[:])
                    stdt4 = pCol.tile([128, 4], F32, tag="lncol",
                                      name=f"stdt4_{p}")
                    nc.scalar.activation(
                        stdt4[:], negv4[:],
                        mybir.ActivationFunctionType.Sqrt,
                        scale=-1.0, bias=eps_t[:])
                    rstd4 = pCol.tile([128, 4], F32, tag="lncol",
                                      name=f"rstd4_{p}")
                    nc.vector.reciprocal(rstd4[:], stdt4[:])
                    for j, lb in enumerate(lbs):
                        eng1 = nc.gpsimd
                        t1 = pOut.tile([128, D], F32, tag="o",
                                       name=f"t1{lb}")
                        eng1.tensor_scalar(t1[:], hhd[lb][:],
                                           mu4[:, j:j + 1],
                                           rstd4[:, j:j + 1], SUB, MUL)
                        if ln_identity:
                            nc.sync.dma_start(
                                out_d[lb * 128:(lb + 1) * 128, :], t1[:])
                        else:
                            tt = pScr.tile([128, D], F32, tag="scr",
                                           name=f"tt{lb}")
                            teng = nc.vector if lb % 2 == 0 else nc.gpsimd
                            teng.tensor_mul(tt[:], t1[:],
                                            stat_b["lnw_row"][:])
                            o_sb = pOut.tile([128, D], F32, tag="o",
                                             name=f"o{lb}")
                            nc.gpsimd.tensor_add(o_sb[:], tt[:],
                                                 stat_b["lnb_row"][:])
                            nc.sync.dma_start(
                                out_d[lb * 128:(lb + 1) * 128, :],
                                o_sb[:])

    nc.compile()
    return nc


_NC_CACHE = {}


def _get_nc(ln_identity=False, bias_zero=False):
    key = (ln_identity, bias_zero)
    if key not in _NC_CACHE:
        _NC_CACHE[key] = _build_program(*key)
    return _NC_CACHE[key]


def kernel(x, adj, pad_mask, W, b, ln_w, ln_b, edge_weight):
    global LAST_RESULT
    x = np.asarray(x, dtype=np.float32)
    adj = np.asarray(adj, dtype=np.float32)
    pad_mask = np.asarray(pad_mask)
    W = np.asarray(W, dtype=np.float32)
    b = np.asarray(b, dtype=np.float32)
    ln_w = np.asarray(ln_w, dtype=np.float32)
    ln_b = np.asarray(ln_b, dtype=np.float32)
    ew = float(np.asarray(edge_weight).reshape(-1)[0])

    ln_identity = bool(np.all(ln_w == 1.0) and np.all(ln_b == 0.0))
    bias_zero = bool(np.all(b == 0.0))
    nc = _get_nc(ln_identity, bias_zero)

    # W.T in d-block-pair DoubleRow layout [2, 128, 2, D]
    wt8 = np.ascontiguousarray(W.T).astype(NPF8)          # [D, D] = [d, o]
    wt8 = np.ascontiguousarray(
        wt8.reshape(2, 2, 128, D).transpose(0, 2, 1, 3))  # [k2, p, s, o]
    ewc = np.full((128, 1), ew * DSCALE, dtype=np.float32)
    ident = np.eye(128, dtype=np.float32)
    b_row_r = b.reshape(1, D).copy()
    ones_row = np.ones((1, 128), dtype=np.float32)
    lnw_row = np.ascontiguousarray(ln_w.reshape(1, D))
    lnb_row = np.ascontiguousarray(ln_b.reshape(1, D))

    in_maps = []
    for c in range(B):
        adjT8 = np.ascontiguousarray(adj[c].T).astype(NPF8)   # [j, i]
        adj8 = np.ascontiguousarray(
            adjT8.reshape(JPN, 2, 128, L).transpose(0, 2, 1, 3))
        x8 = np.ascontiguousarray(
            x[c].reshape(JPN, 2, 128, D).transpose(2, 0, 1, 3)
            * np.float32(D ** 0.5)).astype(ml_dtypes.bfloat16)
        valid = (~pad_mask[c]).astype(np.float32)
        validc = np.ascontiguousarray(valid.reshape(JBN, 128).T)
        # [128, JPN, 2, 128]: valid[(2q+s)*128+p] replicated along last axis
        vrep = np.broadcast_to(
            valid.reshape(JPN, 2, 128).transpose(2, 0, 1)[:, :, :, None],
            (128, JPN, 2, 128))
        in_maps.append({
            "adj8": adj8,
            "x_in": x8,
            "validrep8": np.ascontiguousarray(vrep).astype(NPF8),
            "validc_f": validc,
            "ewc": ewc,
            "wt8": wt8,
            "b_row_r": b_row_r,
            "ones_row": ones_row,
            "lnw_row": lnw_row,
            "lnb_row": lnb_row,
            "ident": ident,
            "identh": ident.astype(ml_dtypes.bfloat16),
        })

    trace = os.environ.get("KERNEL_TRACE", "0") == "1"
    res = run_bass_kernel_spmd(nc, in_maps, core_ids=list(range(B)), trace=trace)
    LAST_RESULT = res
    out = np.stack([res.results[c]["out_t"] for c in range(B)], axis=0)
    return out
